# revision 5
# baseline (speedup 1.0000x reference)
"""Trainium2 Bass kernel for nn_CustomLoss_69999376990919.

Math: the reference's A-inner-product modified Gram-Schmidt + projection
collapses to per-sample 4x4 Gram matrices
    G[s] = P_s diag(c_s) P_s^T,   R[s] = P_s diag(c_s) T_s
after which   loss = mean_s (4 - h^2 tr(R^T G^{-1} R)) / 4
(Cholesky of G == Gram-Schmidt in exact arithmetic; <v,Av> > 0 always holds
since coefficients > 0).  With q = sqrt(c) * p and u = sqrt(c) * t this is
G = q q^T and R = q u^T, so only two scaled bf16 operands are needed.
The device streams all inputs (memory-bound) and produces G/R; the tiny
4x4 solves run on the host in float64.

Sharding: pure data parallelism, batch axis 0 split across 8 cores
(64 samples each), 4 groups of 16 samples per core.

Perf: all HBM loads are plain fp32 on the two HWDGE rings (SP + ACT,
round-robin) — the SWDGE cast-DMA path the previous version used caps at
~170 GB/s and was 95% busy; HWDGE runs at HBM line rate.  fp32->bf16
conversion happens on-chip: ScalarE computes sqrt(c), DVE multiplies
p/t chunks by it (stride-0 broadcast APs) writing bf16.  Layout
n = p*128 + f (p = SBUF partition, f = free chunk); per f a matmul pair
accumulates G and R for the group's 16 samples jointly:
  lhsT = q(f) as [128, (j,s)] stationary, rhs = q(f)/u(f) moving,
  PSUM[(j,s), (s',x)] accumulated over the 128 f-chunks; the s==s' block
diagonals are the per-sample G/R entries (extracted on host).
"""

import numpy as np

from contextlib import ExitStack

import concourse.bacc as bacc
import concourse.tile as tile
from concourse import mybir
from concourse.bass_utils import run_bass_kernel_spmd

B, C, N = 512, 4, 16384
H = 0.0078125  # grid spacing; A = diag(h^2 * coefficients)
NCORES = 8
SPC = B // NCORES  # 64 samples per core
GS = 16            # samples per group
NG = SPC // GS     # 4 groups per core
P = 128            # SBUF partitions; n = p*128 + f
F = N // P         # 128 f-chunks
SCH = 4            # samples per DMA/cast staging chunk (1 MB DMAs)
NCH = GS // SCH
QP = C * GS        # psum partitions (j, s) = 64
GW = 2 * C * GS    # out columns per group (G block + R block)

_CACHE = {}


def _bcast(ap, axis_pos, size):
    """Insert a stride-0 (broadcast) axis into an AP at free-dim position."""
    lay = [list(d) for d in ap.ap]
    lay.insert(axis_pos, [0, size])
    return type(ap)(ap.tensor, ap.offset, lay)


def _build_bass():
    nc = bacc.Bacc(trn_type="TRN2")
    coeff = nc.dram_tensor("coeff", [SPC, N], mybir.dt.float32, kind="ExternalInput")
    preds = nc.dram_tensor("preds", [SPC, C, N], mybir.dt.float32, kind="ExternalInput")
    targs = nc.dram_tensor("targs", [SPC, N, C], mybir.dt.float32, kind="ExternalInput")
    out = nc.dram_tensor("gr_out", [QP, NG * GW], mybir.dt.float32, kind="ExternalOutput")

    coeff_v = coeff[:].rearrange("s (p f) -> p s f", p=P)
    preds_v = preds[:].rearrange("s j (p f) -> p s j f", p=P)
    targs_v = targs[:].rearrange("s (p f) m -> p s f m", p=P)

    # round-robin the loads over the two HWDGE rings (SP + ACT)
    rings = [nc.sync, nc.scalar]
    ring_i = [0]

    def load(out_ap, in_ap):
        rings[ring_i[0] % 2].dma_start(out=out_ap, in_=in_ap)
        ring_i[0] += 1

    with tile.TileContext(nc) as tc, ExitStack() as ctx:
        a32s = ctx.enter_context(tc.tile_pool(name="a32s", bufs=2))
        sa32s = ctx.enter_context(tc.tile_pool(name="sa32s", bufs=2))
        p32s = ctx.enter_context(tc.tile_pool(name="p32s", bufs=3))
        t32s = ctx.enter_context(tc.tile_pool(name="t32s", bufs=3))
        q16s = ctx.enter_context(tc.tile_pool(name="q16s", bufs=2))
        u16s = ctx.enter_context(tc.tile_pool(name="u16s", bufs=2))
        outs = ctx.enter_context(tc.tile_pool(name="outs", bufs=1))
        psums = ctx.enter_context(tc.tile_pool(name="psums", bufs=4, space="PSUM"))

        out_stage = outs.tile([QP, NG * GW], mybir.dt.float32)

        for g in range(NG):
            sl = slice(g * GS, (g + 1) * GS)

            a32 = a32s.tile([P, GS, F], mybir.dt.float32, tag="a32")
            load(a32[:], coeff_v[:, sl, :])
            sa32 = sa32s.tile([P, GS, F], mybir.dt.float32, tag="sa32")
            nc.scalar.sqrt(sa32[:], a32[:])

            q16 = q16s.tile([P, GS, C, F], mybir.dt.bfloat16, tag="q16")
            for ch in range(NCH):
                s0 = g * GS + ch * SCH
                cs = slice(ch * SCH, (ch + 1) * SCH)
                p32 = p32s.tile([P, SCH, C, F], mybir.dt.float32, tag="p32")
                load(p32[:], preds_v[:, s0 : s0 + SCH, :, :])
                # q[p, s, j, f] = p32[p, s, j, f] * sa[p, s, f]  (bcast over j)
                nc.vector.tensor_mul(
                    q16[:, cs, :, :], p32[:], _bcast(sa32[:, cs, :], 2, C)
                )

            # u stored [P, F, GS, C] so the R matmul's moving AP (fixed f)
            # merges to a single contiguous free dim (walrus requirement)
            u16 = u16s.tile([P, F, GS, C], mybir.dt.bfloat16, tag="u16")
            for ch in range(NCH):
                s0 = g * GS + ch * SCH
                cs = slice(ch * SCH, (ch + 1) * SCH)
                t32 = t32s.tile([P, SCH, F, C], mybir.dt.float32, tag="t32")
                load(t32[:], targs_v[:, s0 : s0 + SCH, :, :])
                # u[p, f, s, m] = t32[p, s, f, m] * sa[p, s, f]  (bcast over m)
                nc.vector.tensor_mul(
                    u16[:, :, cs, :].rearrange("p f s m -> p s f m"),
                    t32[:],
                    _bcast(sa32[:, cs, :], 3, C),
                )

            psum_g = psums.tile([QP, GS * C], mybir.dt.float32, tag="pg")
            psum_r = psums.tile([QP, GS * C], mybir.dt.float32, tag="pr")

            # stationary and moving share the (s,j) ordering — both merge to
            # one free dim; PSUM rows/cols are (s*4+j), host takes the 4x4
            # diagonal blocks
            for f in range(F):
                nc.tensor.matmul(
                    psum_g[:],
                    q16[:, :, :, f],  # [128, (s,j)] stationary
                    q16[:, :, :, f],  # [128, (s,j)] moving
                    start=(f == 0),
                    stop=(f == F - 1),
                )
            for f in range(F):
                nc.tensor.matmul(
                    psum_r[:],
                    q16[:, :, :, f],
                    u16[:, f, :, :],  # [128, (s,m)] moving
                    start=(f == 0),
                    stop=(f == F - 1),
                )

            nc.scalar.copy(
                out=out_stage[:, g * GW : g * GW + QP], in_=psum_g[:]
            )
            nc.scalar.copy(
                out=out_stage[:, g * GW + QP : (g + 1) * GW], in_=psum_r[:]
            )
            # drain this group's results while the next group computes
            nc.scalar.dma_start(
                out=out[:, g * GW : (g + 1) * GW],
                in_=out_stage[:, g * GW : (g + 1) * GW],
            )

    if not nc.is_finalized():
        nc.finalize()
    return nc


def _get_nc():
    if "nc" not in _CACHE:
        _CACHE["nc"] = _build_bass()
    return _CACHE["nc"]


def kernel(coefficients, predictions, targets):
    co = np.ascontiguousarray(np.asarray(coefficients, dtype=np.float32))
    pr = np.ascontiguousarray(np.asarray(predictions, dtype=np.float32))
    tg = np.ascontiguousarray(np.asarray(targets, dtype=np.float32))
    assert co.shape == (B, N) and pr.shape == (B, C, N) and tg.shape == (B, N, C)

    nc = _get_nc()
    in_maps = []
    for c in range(NCORES):
        sl = slice(c * SPC, (c + 1) * SPC)
        in_maps.append({"coeff": co[sl], "preds": pr[sl], "targs": tg[sl]})

    res = run_bass_kernel_spmd(nc, in_maps, core_ids=list(range(NCORES)))
    _CACHE["last"] = res

    # host epilogue: extract per-sample 4x4 G/R block diagonals, fp64 solve
    G = np.empty((B, C, C), np.float64)
    R = np.empty((B, C, C), np.float64)
    for c in range(NCORES):
        o = np.asarray(res.results[c]["gr_out"], dtype=np.float64)
        for g in range(NG):
            bg = o[:, g * GW : g * GW + QP].reshape(GS, C, GS, C)
            br = o[:, g * GW + QP : (g + 1) * GW].reshape(GS, C, GS, C)
            s0 = c * SPC + g * GS
            G[s0 : s0 + GS] = np.einsum("sjsk->sjk", bg)
            R[s0 : s0 + GS] = np.einsum("sjsm->sjm", br)

    G = 0.5 * (G + np.swapaxes(G, 1, 2))
    Xs = np.linalg.solve(G, R)
    val = (H * H) * np.einsum("bim,bim->b", R, Xs)
    loss = np.mean((4.0 - val) / 4.0)
    return np.float32(loss)


# revision 7
# speedup vs baseline: 1.0078x; 1.0078x over previous
"""Trainium2 Bass kernel for nn_CustomLoss_69999376990919.

Math: the reference's A-inner-product modified Gram-Schmidt + projection
collapses to per-sample 4x4 Gram matrices
    G[s] = P_s diag(c_s) P_s^T,   R[s] = P_s diag(c_s) T_s
after which   loss = mean_s (4 - h^2 tr(R^T G^{-1} R)) / 4
(Cholesky of G == Gram-Schmidt in exact arithmetic; <v,Av> > 0 always holds
since coefficients > 0).  With q = sqrt(c) * p and u = sqrt(c) * t this is
G = q q^T and R = q u^T, so only two scaled bf16 operands are needed.
The device streams all inputs (memory-bound) and produces G/R; the tiny
4x4 solves run on the host in float64.

Sharding: pure data parallelism, batch axis 0 split across 8 cores
(64 samples each), 2 groups of 32 samples per core.

Perf notes:
- All HBM loads are plain fp32 on the two HWDGE rings (SP + ACT,
  round-robin) — the SWDGE cast-DMA path caps at ~170 GB/s; HWDGE
  sustains ~320+ GB/s.  fp32->bf16 happens on-chip: ScalarE computes
  sqrt(c) once, DVE multiplies p/t by it (stride-0 broadcast APs).
- q/u live in SBUF as [P, F, GS*C]: at fixed f both matmul operands are
  contiguous [128, 128] bf16, so fast-weight-load engages (strided
  weights measured 216 ns/LDWEIGHTS vs ~50 ns contiguous).
- t arrives in f-quarters so the R matmuls chase the DMA; only the last
  quarter's multiply + 32 matmuls are exposed at the end.
Layout: n = p*128 + f (p = SBUF partition, f = free chunk).  Per f a
matmul pair accumulates G and R for all 32 samples jointly:
  PSUM[(s,j), (s',x)] over the 128 f-chunks; the s==s' 4x4 diagonal
blocks are the per-sample G/R entries (extracted on host).
"""

import numpy as np

from contextlib import ExitStack

import concourse.bacc as bacc
import concourse.tile as tile
from concourse import mybir
from concourse.bass_utils import run_bass_kernel_spmd

B, C, N = 512, 4, 16384
H = 0.0078125  # grid spacing; A = diag(h^2 * coefficients)
NCORES = 8
SPC = B // NCORES  # 64 samples per core
GS = 32            # samples per group
NG = SPC // GS     # 2 groups per core
P = 128            # SBUF partitions; n = p*128 + f
F = N // P         # 128 f-chunks
SCH = 4            # samples per p-DMA chunk (1 MB DMAs)
NCH = GS // SCH
TQ = 4             # t-quarters per group
FQ = F // TQ
QP = C * GS        # psum partitions (s, j) = 128
GW = 2 * QP        # out columns per group (G block + R block)

_CACHE = {}


def _bcast(ap, axis_pos, size):
    """Insert a stride-0 (broadcast) axis into an AP at the given position."""
    lay = [list(d) for d in ap.ap]
    lay.insert(axis_pos, [0, size])
    return type(ap)(ap.tensor, ap.offset, lay)


def _build_bass():
    nc = bacc.Bacc(trn_type="TRN2")
    coeff = nc.dram_tensor("coeff", [SPC, N], mybir.dt.float32, kind="ExternalInput")
    preds = nc.dram_tensor("preds", [SPC, C, N], mybir.dt.float32, kind="ExternalInput")
    targs = nc.dram_tensor("targs", [SPC, N, C], mybir.dt.float32, kind="ExternalInput")
    out = nc.dram_tensor("gr_out", [QP, NG * GW], mybir.dt.float32, kind="ExternalOutput")

    coeff_v = coeff[:].rearrange("s (p f) -> p s f", p=P)
    preds_v = preds[:].rearrange("s j (p f) -> p s j f", p=P)
    targs_v = targs[:].rearrange("s (p f) m -> p s f m", p=P)

    # round-robin the loads over the two HWDGE rings (SP + ACT)
    rings = [nc.sync, nc.scalar]
    ring_i = [0]

    def load(out_ap, in_ap):
        rings[ring_i[0] % 2].dma_start(out=out_ap, in_=in_ap)
        ring_i[0] += 1

    with tile.TileContext(nc) as tc, ExitStack() as ctx:
        a32s = ctx.enter_context(tc.tile_pool(name="a32s", bufs=2))
        sa16s = ctx.enter_context(tc.tile_pool(name="sa16s", bufs=2))
        p32s = ctx.enter_context(tc.tile_pool(name="p32s", bufs=2))
        t32s = ctx.enter_context(tc.tile_pool(name="t32s", bufs=2))
        q16s = ctx.enter_context(tc.tile_pool(name="q16s", bufs=2))
        u16s = ctx.enter_context(tc.tile_pool(name="u16s", bufs=4))
        outs = ctx.enter_context(tc.tile_pool(name="outs", bufs=1))
        psums = ctx.enter_context(tc.tile_pool(name="psums", bufs=4, space="PSUM"))

        out_stage = outs.tile([QP, NG * GW], mybir.dt.float32)

        for g in range(NG):
            sl = slice(g * GS, (g + 1) * GS)

            a32 = a32s.tile([P, GS, F], mybir.dt.float32, tag="a32")
            load(a32[:], coeff_v[:, sl, :])
            sa16 = sa16s.tile([P, GS, F], mybir.dt.bfloat16, tag="sa16")
            nc.scalar.sqrt(sa16[:], a32[:])

            # q stored [P, F, GS, C]: at fixed f the (s,j) block is one
            # contiguous [128, 128] — required for fast weight load and
            # single-free-dim matmul APs.
            q16 = q16s.tile([P, F, GS, C], mybir.dt.bfloat16, tag="q16")
            for ch in range(NCH):
                s0 = g * GS + ch * SCH
                cs = slice(ch * SCH, (ch + 1) * SCH)
                p32 = p32s.tile([P, SCH, C, F], mybir.dt.float32, tag="p32")
                load(p32[:], preds_v[:, s0 : s0 + SCH, :, :])
                # q[p, f, s, j] = p32[p, s, j, f] * sa[p, s, f]  (bcast over j)
                # iterate (f, s, j): output writes land in 32 B contiguous
                # runs (fully-strided DVE writes measured 4.3x slower)
                nc.vector.tensor_mul(
                    q16[:, :, cs, :],
                    p32[:].rearrange("p s j f -> p f s j"),
                    _bcast(sa16[:, cs, :].rearrange("p s f -> p f s"), 3, C),
                )

            psum_g = psums.tile([QP, GS * C], mybir.dt.float32, tag="pg")
            psum_r = psums.tile([QP, GS * C], mybir.dt.float32, tag="pr")

            for f in range(F):
                nc.tensor.matmul(
                    psum_g[:],
                    q16[:, f, :, :],  # [128, 128] contiguous, stationary
                    q16[:, f, :, :],  # moving
                    start=(f == 0),
                    stop=(f == F - 1),
                )

            # t arrives in f-quarters; R matmuls chase each quarter
            for tq in range(TQ):
                fs = slice(tq * FQ, (tq + 1) * FQ)
                t32 = t32s.tile([P, GS, FQ, C], mybir.dt.float32, tag="t32")
                load(t32[:], targs_v[:, sl, fs, :])
                u16 = u16s.tile([P, FQ, GS, C], mybir.dt.bfloat16, tag="u16")
                # u[p, f, s, m] = t32[p, s, f, m] * sa[p, s, f]  (bcast over m)
                nc.vector.tensor_mul(
                    u16[:].rearrange("p f s m -> p s f m"),
                    t32[:],
                    _bcast(sa16[:, :, fs], 3, C),
                )
                for fo in range(FQ):
                    f = tq * FQ + fo
                    nc.tensor.matmul(
                        psum_r[:],
                        q16[:, f, :, :],
                        u16[:, fo, :, :],
                        start=(f == 0),
                        stop=(f == F - 1),
                    )

            nc.scalar.copy(out=out_stage[:, g * GW : g * GW + QP], in_=psum_g[:])
            nc.scalar.copy(
                out=out_stage[:, g * GW + QP : (g + 1) * GW], in_=psum_r[:]
            )
            # drain this group's results while the next group computes
            nc.scalar.dma_start(
                out=out[:, g * GW : (g + 1) * GW],
                in_=out_stage[:, g * GW : (g + 1) * GW],
            )

    if not nc.is_finalized():
        nc.finalize()
    return nc


def _get_nc():
    if "nc" not in _CACHE:
        _CACHE["nc"] = _build_bass()
    return _CACHE["nc"]


def kernel(coefficients, predictions, targets):
    co = np.ascontiguousarray(np.asarray(coefficients, dtype=np.float32))
    pr = np.ascontiguousarray(np.asarray(predictions, dtype=np.float32))
    tg = np.ascontiguousarray(np.asarray(targets, dtype=np.float32))
    assert co.shape == (B, N) and pr.shape == (B, C, N) and tg.shape == (B, N, C)

    nc = _get_nc()
    in_maps = []
    for c in range(NCORES):
        sl = slice(c * SPC, (c + 1) * SPC)
        in_maps.append({"coeff": co[sl], "preds": pr[sl], "targs": tg[sl]})

    res = run_bass_kernel_spmd(nc, in_maps, core_ids=list(range(NCORES)))
    _CACHE["last"] = res

    # host epilogue: extract per-sample 4x4 G/R diagonal blocks, fp64 solve
    G = np.empty((B, C, C), np.float64)
    R = np.empty((B, C, C), np.float64)
    for c in range(NCORES):
        o = np.asarray(res.results[c]["gr_out"], dtype=np.float64)
        for g in range(NG):
            bg = o[:, g * GW : g * GW + QP].reshape(GS, C, GS, C)
            br = o[:, g * GW + QP : (g + 1) * GW].reshape(GS, C, GS, C)
            s0 = c * SPC + g * GS
            G[s0 : s0 + GS] = np.einsum("sjsk->sjk", bg)
            R[s0 : s0 + GS] = np.einsum("sjsm->sjm", br)

    G = 0.5 * (G + np.swapaxes(G, 1, 2))
    Xs = np.linalg.solve(G, R)
    val = (H * H) * np.einsum("bim,bim->b", R, Xs)
    loss = np.mean((4.0 - val) / 4.0)
    return np.float32(loss)


# revision 11
# speedup vs baseline: 1.0289x; 1.0209x over previous
"""Trainium2 Bass kernel for nn_CustomLoss_69999376990919.

Math: the reference's A-inner-product modified Gram-Schmidt + projection
collapses to per-sample 4x4 Gram matrices
    G[s] = P_s diag(c_s) P_s^T,   R[s] = P_s diag(c_s) T_s
after which   loss = mean_s (4 - h^2 tr(R^T G^{-1} R)) / 4
(Cholesky of G == Gram-Schmidt in exact arithmetic; <v,Av> > 0 always
holds since coefficients > 0).  With q = sqrt(c) * p and u = sqrt(c) * t
this is G = q q^T and R = q u^T.  The device streams all inputs
(memory-bound) and produces G/R; the tiny 4x4 solves run on the host in
float64.

Sharding: pure data parallelism, batch axis 0 split across 8 cores
(64 samples each), processed as 2 halves of 32 samples.

Perf notes:
- Loads are plain fp32 spread over three DMA paths: the two HWDGE rings
  (SP + ACT) plus the SWDGE ring (gpsimd) — each 512B-descriptor path
  alone caps near ~290 GB/s; the third path targets the ~358 GB/s HBM
  limit.  No cast-DMAs (those cap at ~170 GB/s total).
- Phase order: all coefficients+predictions first (q + all G matmuls
  complete early), then targets stream in f-quarters with the R matmuls
  chasing each quarter — only the last quarter's multiply + 32 matmuls
  are exposed at the end.
- q/u live in SBUF as [P, F, GS*C]: at fixed f both matmul operands are
  contiguous [128, 128] bf16 (fast weight load; strided weights measured
  216 ns/LDWEIGHTS vs ~100 ns, strided DVE writes 4.3x slower).
- fp32->bf16 happens on-chip: ScalarE computes sqrt(c), DVE multiplies
  p/t by it (stride-0 broadcast APs, iteration chosen so writes land in
  >=32 B runs).
Layout: n = p*128 + f (p = SBUF partition, f = free chunk).  Per f a
matmul pair accumulates G and R for 32 samples jointly:
  PSUM[(s,j), (s',x)] over the 128 f-chunks; the s==s' 4x4 diagonal
blocks are the per-sample G/R entries (extracted on host).
"""

import numpy as np

from contextlib import ExitStack

import concourse.bacc as bacc
import concourse.tile as tile
from concourse import mybir
from concourse.bass_utils import run_bass_kernel_spmd

B, C, N = 512, 4, 16384
H = 0.0078125  # grid spacing; A = diag(h^2 * coefficients)
NCORES = 8
SPC = B // NCORES  # 64 samples per core
GS = 32            # samples per half
NH = SPC // GS     # 2 halves per core
P = 128            # SBUF partitions; n = p*128 + f
F = N // P         # 128 f-chunks
SCH = 4            # samples per p-DMA chunk (1 MB DMAs)
NCH = GS // SCH
ACH = 16           # samples per a-DMA chunk (1 MB DMAs)
TQ = 4             # t-quarters per half
FQ = F // TQ
QP = C * GS        # psum partitions (s, j) = 128
OW = 4 * QP        # out columns: [G(h0) | R(h0) | G(h1) | R(h1)]

_CACHE = {}


def _bcast(ap, axis_pos, size):
    """Insert a stride-0 (broadcast) axis into an AP at the given position."""
    lay = [list(d) for d in ap.ap]
    lay.insert(axis_pos, [0, size])
    return type(ap)(ap.tensor, ap.offset, lay)


def _build_bass():
    nc = bacc.Bacc(trn_type="TRN2")
    coeff = nc.dram_tensor("coeff", [SPC, N], mybir.dt.float32, kind="ExternalInput")
    preds = nc.dram_tensor("preds", [SPC, C, N], mybir.dt.float32, kind="ExternalInput")
    targs = nc.dram_tensor("targs", [SPC, N, C], mybir.dt.float32, kind="ExternalInput")
    out = nc.dram_tensor("gr_out", [QP, OW], mybir.dt.float32, kind="ExternalOutput")

    coeff_v = coeff[:].rearrange("s (p f) -> p s f", p=P)
    preds_v = preds[:].rearrange("s j (p f) -> p s j f", p=P)
    targs_v = targs[:].rearrange("s (p f) m -> p s f m", p=P)

    # round-robin fp32 loads over the two HWDGE rings (SP + ACT)
    rings = [nc.sync, nc.scalar]
    ring_i = [0]

    def load(out_ap, in_ap):
        rings[ring_i[0] % 2].dma_start(out=out_ap, in_=in_ap)
        ring_i[0] += 1

    with tile.TileContext(nc) as tc, ExitStack() as ctx:
        a32s = ctx.enter_context(tc.tile_pool(name="a32s", bufs=2))
        sa16s = ctx.enter_context(tc.tile_pool(name="sa16s", bufs=2))
        p32s = ctx.enter_context(tc.tile_pool(name="p32s", bufs=3))
        t32s = ctx.enter_context(tc.tile_pool(name="t32s", bufs=3))
        q16s = ctx.enter_context(tc.tile_pool(name="q16s", bufs=2))
        u16s = ctx.enter_context(tc.tile_pool(name="u16s", bufs=3))
        outs = ctx.enter_context(tc.tile_pool(name="outs", bufs=1))
        psums = ctx.enter_context(tc.tile_pool(name="psums", bufs=4, space="PSUM"))

        out_stage = outs.tile([QP, OW], mybir.dt.float32)

        # ---- phase A: coefficients (SWDGE) + predictions (HWDGE) --------
        sa16 = [None] * NH
        for h in range(NH):
            sa16[h] = sa16s.tile(
                [P, GS, F], mybir.dt.bfloat16, tag="sa16", name=f"sa16_{h}"
            )
            for ac in range(GS // ACH):
                s0 = h * GS + ac * ACH
                al = slice(ac * ACH, (ac + 1) * ACH)
                a32 = a32s.tile([P, ACH, F], mybir.dt.float32, tag="a32")
                nc.gpsimd.dma_start(out=a32[:], in_=coeff_v[:, s0 : s0 + ACH, :])
                nc.scalar.sqrt(sa16[h][:, al, :], a32[:])

        q16 = [None] * NH
        for h in range(NH):
            q16[h] = q16s.tile(
                [P, F, GS, C], mybir.dt.bfloat16, tag="q16", name=f"q16_{h}"
            )
            for ch in range(NCH):
                s0 = h * GS + ch * SCH
                cs = slice(ch * SCH, (ch + 1) * SCH)
                p32 = p32s.tile([P, SCH, C, F], mybir.dt.float32, tag="p32")
                load(p32[:], preds_v[:, s0 : s0 + SCH, :, :])
                # q[p, f, s, j] = p32[p, s, j, f] * sa[p, s, f]  (bcast j);
                # iterate (f, s, j) so writes land in 32 B contiguous runs
                nc.vector.tensor_mul(
                    q16[h][:, :, cs, :],
                    p32[:].rearrange("p s j f -> p f s j"),
                    _bcast(sa16[h][:, cs, :].rearrange("p s f -> p f s"), 3, C),
                )

        psum_g = [
            psums.tile([QP, GS * C], mybir.dt.float32, tag="pg", name=f"pg_{h}")
            for h in range(NH)
        ]
        psum_r = [
            psums.tile([QP, GS * C], mybir.dt.float32, tag="pr", name=f"pr_{h}")
            for h in range(NH)
        ]

        for h in range(NH):
            for f in range(F):
                nc.tensor.matmul(
                    psum_g[h][:],
                    q16[h][:, f, :, :],  # [128, 128] contiguous, stationary
                    q16[h][:, f, :, :],  # moving
                    start=(f == 0),
                    stop=(f == F - 1),
                )
            nc.scalar.copy(
                out=out_stage[:, (2 * h) * QP : (2 * h + 1) * QP], in_=psum_g[h][:]
            )

        # ---- phase B: targets in f-quarters, R matmuls chase ------------
        for tq in range(TQ):
            fs = slice(tq * FQ, (tq + 1) * FQ)
            for h in range(NH):
                sl = slice(h * GS, (h + 1) * GS)
                t32 = t32s.tile([P, GS, FQ, C], mybir.dt.float32, tag="t32")
                # alternate the three rings: h0 quarters on HWDGE, h1 on SWDGE
                if h == 0:
                    load(t32[:], targs_v[:, sl, fs, :])
                else:
                    nc.gpsimd.dma_start(out=t32[:], in_=targs_v[:, sl, fs, :])
                u16 = u16s.tile([P, FQ, GS, C], mybir.dt.bfloat16, tag="u16")
                # u[p, f, s, m] = t32[p, s, f, m] * sa[p, s, f]  (bcast m)
                nc.vector.tensor_mul(
                    u16[:].rearrange("p f s m -> p s f m"),
                    t32[:],
                    _bcast(sa16[h][:, :, fs], 3, C),
                )
                for fo in range(FQ):
                    f = tq * FQ + fo
                    nc.tensor.matmul(
                        psum_r[h][:],
                        q16[h][:, f, :, :],
                        u16[:, fo, :, :],
                        start=(f == 0),
                        stop=(f == F - 1),
                    )

        for h in range(NH):
            nc.scalar.copy(
                out=out_stage[:, (2 * h + 1) * QP : (2 * h + 2) * QP],
                in_=psum_r[h][:],
            )
        nc.scalar.dma_start(out=out[:], in_=out_stage[:])

    if not nc.is_finalized():
        nc.finalize()
    return nc


def _get_nc():
    if "nc" not in _CACHE:
        _CACHE["nc"] = _build_bass()
    return _CACHE["nc"]


def kernel(coefficients, predictions, targets):
    co = np.ascontiguousarray(np.asarray(coefficients, dtype=np.float32))
    pr = np.ascontiguousarray(np.asarray(predictions, dtype=np.float32))
    tg = np.ascontiguousarray(np.asarray(targets, dtype=np.float32))
    assert co.shape == (B, N) and pr.shape == (B, C, N) and tg.shape == (B, N, C)

    nc = _get_nc()
    in_maps = []
    for c in range(NCORES):
        sl = slice(c * SPC, (c + 1) * SPC)
        in_maps.append({"coeff": co[sl], "preds": pr[sl], "targs": tg[sl]})

    res = run_bass_kernel_spmd(nc, in_maps, core_ids=list(range(NCORES)))
    _CACHE["last"] = res

    # host epilogue: extract per-sample 4x4 G/R diagonal blocks, fp64 solve
    G = np.empty((B, C, C), np.float64)
    R = np.empty((B, C, C), np.float64)
    for c in range(NCORES):
        o = np.asarray(res.results[c]["gr_out"], dtype=np.float64)
        for h in range(NH):
            bg = o[:, (2 * h) * QP : (2 * h + 1) * QP].reshape(GS, C, GS, C)
            br = o[:, (2 * h + 1) * QP : (2 * h + 2) * QP].reshape(GS, C, GS, C)
            s0 = c * SPC + h * GS
            G[s0 : s0 + GS] = np.einsum("sjsk->sjk", bg)
            R[s0 : s0 + GS] = np.einsum("sjsm->sjm", br)

    G = 0.5 * (G + np.swapaxes(G, 1, 2))
    Xs = np.linalg.solve(G, R)
    val = (H * H) * np.einsum("bim,bim->b", R, Xs)
    loss = np.mean((4.0 - val) / 4.0)
    return np.float32(loss)
